# revision 15
# baseline (speedup 1.0000x reference)
"""EnhancedSparseFeatureAttention Trainium2 kernel.

Data-parallel over batch across 8 NeuronCores (32 batches/core), parameters
replicated. Hardcoded for B=256, F=128, D=512, H=8, HD=64, top-k=64.

Per-core pipeline (per batch b):
  - xT = transpose(x_b) via PE (4x 128x128 transposes)
  - qT = (Wq/8)-chunks.T @ xT  (+ bq/8 via K=1 bias-matmul)   [d', f] layout
    kT likewise; v = xT-chunks.T @ Wv (+ bv)                  [f, d'] layout
  - per head: scores = qT_h.T @ kT_h + prior_b (all in PSUM; prior added by a
    prior_b.T @ I matmul into the same accumulation group)
  - top-64 threshold per row: 8 chained MAX8_DISPLACE_ANT passes (custom DVE
    uop program: the 8-slice MIN/swap cascade streams out the input multiset
    with its top-8 removed, and appends min(top-8); 8 rounds leave the 64th
    largest in the tail slot)
  - p = exp(scores) * (scores >= thresh), denom = row-sum: one fused
    MASKED_EXP_SUM_ANT custom-DVE pass (exact ties semantics of the reference)
  - attn = p / denom (gpsimd); attnT via PE transpose; outT_h = v_h @ attnT
  - o = outT-chunks.T @ Wo (+ bo), DMA out
prior_b = corr_prior * (fi x fi) + pv_b * 0.5 * xor-block-mask, where pv is
the price/vol detector MLP computed for all 32 batches up front on PE/ACT.
"""

from dataclasses import dataclass
import os

import numpy as np

import concourse.bacc as bacc
import concourse.bass as bass
import concourse.dve_ops as dve_ops
import concourse.mybir as mybir
import concourse.tile as tile
from concourse.bass_utils import run_bass_kernel_spmd
from concourse.dve_ops import OPS, CUSTOM_DVE_SPECS, DveOp, _SUB_OPCODE_FOR_NAME
from concourse.dve_spec import C0, Spec, Src0, Src1, Zero, lower, select
from concourse.dve_uop import (
    AluInp,
    AluOp as UAluOp,
    DveOpSpec,
    ENABLE,
    InpSel,
    OutPath,
    OutSel,
    Trigger,
    UopConfig,
)
from concourse.masks import make_identity

B, F, D, H = 256, 128, 512, 8
HD = D // H          # 64
NCORES = 8
BB = B // NCORES     # 32 batches per core
DC = D // 128        # 4 d-chunks
f32 = mybir.dt.float32
f32r = mybir.dt.float32r
AF = mybir.ActivationFunctionType
ALU = mybir.AluOpType
FLT_MIN = np.float32(np.finfo(np.float32).min)

# --------------------------------------------------------------------------
# Custom DVE ops
# --------------------------------------------------------------------------


def _displace_ref(in0, in1, c0, c1, c2):
    """CoreSim model of the displace ops. Matches the HW output structure:
    the first 8 outputs are the displaced -FLT_MAX inits (element i < 8
    always meets a remaining init in the cascade), followed by the N-8
    surviving values (survivor order is unspecified on HW; downstream use
    is set-based), then min(top-8). Handles [P, N] and [P, S, N]."""
    x = np.array(in0, dtype=np.float32, copy=True)
    shp = x.shape
    x = x.reshape(-1, shp[-1])
    n = x.shape[1]
    idx = np.argsort(-x, axis=1, kind="stable")
    kth = np.take_along_axis(x, idx[:, 7:8], axis=1)
    survivors = np.take_along_axis(x, np.sort(idx[:, 8:], axis=1), axis=1)
    fill = np.full((x.shape[0], 8), FLT_MIN, np.float32)
    r = np.concatenate([fill, survivors, kth], axis=1)
    return r.reshape(*shp[:-1], shp[-1] + 1)


def _build_max8_displace_spec(opcode: int) -> DveOpSpec:
    # uop[0] INIT: one dummy element carries MAX_NEG down delay lane 0;
    # BYPASS ALUs with swap_enable latch it into every slice's swap flop
    # (for BYPASS(a,b): alu_out = a, swap <- b).
    init = UopConfig()
    init.enable_input(InpSel.MAX_NEG, 1)  # lane 1 -> stage0 PREV_DELAY_0
    init.repeat_count = 1
    init.trigger = (Trigger.COUNT, Trigger.NONE, Trigger.NONE)
    init.next_uop = (1, 0, 0)
    for k in range(8):
        blk = init.datapath_config[k]
        blk.enable_alu(UAluOp.BYPASS, AluInp.PREV_ALU_OUT, AluInp.PREV_DELAY_0)
        blk.swap_enable = ENABLE
        blk.pass_through_delay(0)

    # uop[1] STEADY: per slice alu_out = min(stream, swap), swap <- max(...).
    # The cascade keeps the running top-8 in the swap flops; slice 7 emits
    # the displaced value each cycle.
    steady = UopConfig()
    steady.enable_input(InpSel.SRC_0, 0)
    steady.require_inp0 = ENABLE
    steady.trigger = (Trigger.SRC_TENSOR_DONE, Trigger.NONE, Trigger.NONE)
    steady.next_uop = (2, 0, 0)
    for k in range(8):
        blk = steady.datapath_config[k]
        blk.enable_alu(UAluOp.MIN, AluInp.PREV_ALU_OUT, AluInp.CURR_SWAP_OUT)
        blk.swap_enable = ENABLE
    steady.enable_output(OutSel.ALU_OUT, OutPath.WR0_LO)

    # uop[2] DRAIN: one +inf element MIN-reduces across the 8 swap flops ->
    # min(top-8) = the 8th largest of this round, appended to the stream.
    drain = UopConfig()
    drain.enable_input(InpSel.POS_INF, 0)
    drain.repeat_count = 1
    drain.trigger = (Trigger.COUNT, Trigger.NONE, Trigger.NONE)
    drain.next_uop = (0, 0, 0)
    for k in range(8):
        blk = drain.datapath_config[k]
        blk.enable_alu(UAluOp.MIN, AluInp.PREV_ALU_OUT, AluInp.CURR_SWAP_OUT)
    drain.enable_output(OutSel.ALU_OUT, OutPath.WR0_LO)

    spec = DveOpSpec(name="MAX8_DISPLACE_ANT", opcode=opcode,
                     uops=[init, steady, drain], rd1_en=False)
    spec.validate("v3")
    return spec


def _build_max8_displace_seg_spec(opcode: int) -> DveOpSpec:
    """Segmented variant: in0 [P, S, N] -> out [P, S, N+1]. The uop FSM loops
    init -> steady -> drain per innermost row (SUB_DIM_DONE), re-latching the
    swap flops between rows, so one instruction processes S rows."""
    def mk_init(next_idx):
        u = UopConfig()
        u.enable_input(InpSel.MAX_NEG, 1)
        u.repeat_count = 1
        u.trigger = (Trigger.COUNT, Trigger.NONE, Trigger.NONE)
        u.next_uop = (next_idx, 0, 0)
        for k in range(8):
            blk = u.datapath_config[k]
            blk.enable_alu(UAluOp.BYPASS, AluInp.PREV_ALU_OUT,
                           AluInp.PREV_DELAY_0)
            blk.swap_enable = ENABLE
            blk.pass_through_delay(0)
        return u

    init0 = mk_init(1)

    steady = UopConfig()
    steady.enable_input(InpSel.SRC_0, 0)
    steady.require_inp0 = ENABLE
    steady.trigger = (Trigger.SUB_DIM_DONE, Trigger.NONE, Trigger.NONE)
    steady.next_uop = (2, 0, 0)
    for k in range(8):
        blk = steady.datapath_config[k]
        blk.enable_alu(UAluOp.MIN, AluInp.PREV_ALU_OUT, AluInp.CURR_SWAP_OUT)
        blk.swap_enable = ENABLE
    steady.enable_output(OutSel.ALU_OUT, OutPath.WR0_LO)

    drain = UopConfig()
    drain.enable_input(InpSel.POS_INF, 0)
    drain.repeat_count = 1
    # last row: source exhausted -> IDLE; otherwise re-init for the next row
    drain.trigger = (Trigger.SRC_TENSOR_DONE, Trigger.COUNT, Trigger.NONE)
    drain.next_uop = (0, 3, 0)
    for k in range(8):
        blk = drain.datapath_config[k]
        blk.enable_alu(UAluOp.MIN, AluInp.PREV_ALU_OUT, AluInp.CURR_SWAP_OUT)
    drain.enable_output(OutSel.ALU_OUT, OutPath.WR0_LO)

    init2 = mk_init(1)

    spec = DveOpSpec(name="MAX8_DISPLACE_SEG_ANT", opcode=opcode,
                     uops=[init0, steady, drain, init2], rd1_en=False)
    spec.validate("v3")
    return spec


@dataclass(frozen=True)
class _HandDveOp(DveOp):
    """DveOp whose uop program is hand-authored (bypasses Spec lowering)."""

    def compile(self, ver):
        assert ver == "v3", f"{self.name} only authored for v3, got {ver}"
        return _HAND_SPECS[self.name]


def _mes_ref(in0, in1, c0, c1, c2):
    out = np.where(in1 >= c0, in0, np.float32(0.0)).astype(np.float32)
    return out, out.sum(axis=1, keepdims=True, dtype=np.float32)


_DVE_EXT = None


def _register_custom_ops():
    global _DVE_EXT, _MAX8_DISPLACE_SPEC
    if _DVE_EXT is not None:
        return _DVE_EXT
    base = dve_ops._CUSTOM_DVE_ROW_BASE

    global _HAND_SPECS
    _HAND_SPECS = {}
    row = base + len(OPS)
    _HAND_SPECS["MAX8_DISPLACE_ANT"] = _build_max8_displace_spec(row)
    disp_spec = Spec(body=Src0, reference=_displace_ref)
    disp = _HandDveOp("MAX8_DISPLACE_ANT", disp_spec, subdim=False, uops_sha={})
    OPS.append(disp)
    _SUB_OPCODE_FOR_NAME["MAX8_DISPLACE_ANT"] = row
    CUSTOM_DVE_SPECS["MAX8_DISPLACE_ANT"] = disp_spec

    rowseg = base + len(OPS)
    _HAND_SPECS["MAX8_DISPLACE_SEG_ANT"] = _build_max8_displace_seg_spec(rowseg)
    dseg_spec = Spec(body=Src0, reference=_displace_ref)
    dseg = _HandDveOp("MAX8_DISPLACE_SEG_ANT", dseg_spec, subdim=True,
                      uops_sha={})
    OPS.append(dseg)
    _SUB_OPCODE_FOR_NAME["MAX8_DISPLACE_SEG_ANT"] = rowseg
    CUSTOM_DVE_SPECS["MAX8_DISPLACE_SEG_ANT"] = dseg_spec

    # out = where(in1 >= s0, in0, 0); accum_out = row-sum(out)
    mes_spec = Spec(body=select(Src1 >= C0, Src0, Zero),
                    accum=UAluOp.ADD, reference=_mes_ref)
    row2 = base + len(OPS)
    _SUB_OPCODE_FOR_NAME["MASKED_EXP_SUM_ANT"] = row2
    tmp = DveOpSpec(name="MASKED_EXP_SUM_ANT", opcode=row2,
                    uops=lower(mes_spec, ver="v3"), rd1_en=True)
    mes = DveOp("MASKED_EXP_SUM_ANT", mes_spec, subdim=False,
                uops_sha={"v3": tmp.sha("v3")})
    OPS.append(mes)
    CUSTOM_DVE_SPECS["MASKED_EXP_SUM_ANT"] = mes_spec

    _DVE_EXT = (disp, mes, dseg)
    return _DVE_EXT


# --------------------------------------------------------------------------
# Kernel build
# --------------------------------------------------------------------------


def _bc(ap, n):
    """Broadcast an AP across n partitions (stride-0 leading dim)."""
    return bass.AP(tensor=ap.tensor, offset=ap.offset, ap=[[0, n]] + list(ap.ap))


def build(bb=BB):
    DISP, MES, DSEG = _register_custom_ops()
    nc = bacc.Bacc("TRN2", target_bir_lowering=False, debug=False)

    x_h = nc.dram_tensor("x", [bb, F, D], f32, kind="ExternalInput")
    wq_h = nc.dram_tensor("Wq", [D, D], f32r, kind="ExternalInput")
    bq_h = nc.dram_tensor("bq", [D], f32, kind="ExternalInput")
    wk_h = nc.dram_tensor("Wk", [D, D], f32r, kind="ExternalInput")
    bk_h = nc.dram_tensor("bk", [D], f32, kind="ExternalInput")
    wv_h = nc.dram_tensor("Wv", [D, D], f32r, kind="ExternalInput")
    bv_h = nc.dram_tensor("bv", [D], f32, kind="ExternalInput")
    wo_h = nc.dram_tensor("Wo", [D, D], f32r, kind="ExternalInput")
    bo_h = nc.dram_tensor("bo", [D], f32, kind="ExternalInput")
    cp_h = nc.dram_tensor("corr_prior", [F, F], f32, kind="ExternalInput")
    fi_h = nc.dram_tensor("feat_imp", [F], f32, kind="ExternalInput")
    w1_h = nc.dram_tensor("W1", [2 * D, D], f32, kind="ExternalInput")
    b1_h = nc.dram_tensor("b1", [D], f32, kind="ExternalInput")
    w2_h = nc.dram_tensor("W2", [D, 1], f32, kind="ExternalInput")
    b2_h = nc.dram_tensor("b2", [1], f32, kind="ExternalInput")
    out_h = nc.dram_tensor("out", [bb, F, D], f32, kind="ExternalOutput")

    with tile.TileContext(nc) as tc:
        with (
            tc.tile_pool(name="singles", bufs=1) as singles,
            tc.tile_pool(name="work", bufs=int(os.environ.get("KX_WORK", "2"))) as sb,
            tc.tile_pool(name="heads", bufs=int(os.environ.get("KX_HD", "2"))) as hd,
            tc.tile_pool(name="vo", bufs=int(os.environ.get("KX_VO", "2"))) as vo,
            tc.tile_pool(name="ps_qk", bufs=int(os.environ.get("KX_PSQK", "1")), space="PSUM") as ps_qk,
            tc.tile_pool(name="ps_voo", bufs=int(os.environ.get("KX_PSVOO", "2")), space="PSUM") as ps_voo,
            tc.tile_pool(name="ps_sc", bufs=int(os.environ.get("KX_PSSC", "1")), space="PSUM") as ps_sc,
            tc.tile_pool(name="ps_at", bufs=int(os.environ.get("KX_PSAT", "1")), space="PSUM") as ps_at,
            tc.tile_pool(name="ps_t", bufs=int(os.environ.get("KX_PST", "1")), space="PSUM") as ps_t,
            tc.tile_pool(name="dram", bufs=1, space="DRAM") as dram,
        ):
            # ---------------- one-time constants ----------------
            identity = singles.tile([128, 128], f32)
            make_identity(nc, identity)

            ones_row = singles.tile([1, 128], f32)
            nc.vector.memset(ones_row, 1.0)
            ones_row_r = singles.tile([1, 128], f32r)
            nc.vector.tensor_copy(ones_row_r, ones_row)

            # weights, [128, chunk, out] layout (contraction dim on partitions)
            # f32r is bit-identical to f32 — DMA weights straight into f32r
            wq_r = singles.tile([128, DC, D], f32r)
            nc.sync.dma_start(wq_r, wq_h[:].rearrange("(c p) e -> p c e", p=128))
            nc.vector.tensor_scalar_mul(wq_r, wq_r, 0.125)  # fold 1/sqrt(HD)
            wk_r = singles.tile([128, DC, D], f32r)
            nc.sync.dma_start(wk_r, wk_h[:].rearrange("(c p) e -> p c e", p=128))
            wv_r = singles.tile([128, DC, D], f32r)
            nc.sync.dma_start(wv_r, wv_h[:].rearrange("(c p) e -> p c e", p=128))
            wo_r = singles.tile([128, DC, D], f32r)
            nc.sync.dma_start(wo_r, wo_h[:].rearrange("(c p) e -> p c e", p=128))
            w1_s = singles.tile([128, 2 * DC, D], f32)
            nc.sync.dma_start(w1_s, w1_h[:].rearrange("(c p) e -> p c e", p=128))
            w2_s = singles.tile([128, DC, 1], f32)
            nc.sync.dma_start(w2_s, w2_h[:].rearrange("(c p) e -> p c e", p=128))

            # bias rows [1, D] (used as K=1 matmul operands)
            bq_col = singles.tile([128, DC], f32)
            nc.sync.dma_start(bq_col, bq_h[:].rearrange("(c p) -> p c", p=128))
            nc.vector.tensor_scalar_mul(bq_col, bq_col, 0.125)
            bk_col = singles.tile([128, DC], f32)
            nc.sync.dma_start(bk_col, bk_h[:].rearrange("(c p) -> p c", p=128))
            bv_row = singles.tile([1, D], f32)
            nc.sync.dma_start(bv_row, _bc(bv_h[:], 1))
            bv_r = singles.tile([1, D], f32r)
            nc.vector.tensor_copy(bv_r, bv_row)
            bo_row = singles.tile([1, D], f32)
            nc.sync.dma_start(bo_row, _bc(bo_h[:], 1))
            bo_r = singles.tile([1, D], f32r)
            nc.vector.tensor_copy(bo_r, bo_row)
            b1_col = singles.tile([128, DC], f32)
            nc.sync.dma_start(b1_col, b1_h[:].rearrange("(c p) -> p c", p=128))
            b2_bc = singles.tile([128, 1], f32)
            nc.sync.dma_start(b2_bc, _bc(b2_h[:], 128))

            # prior = corr_prior * fi[i] * fi[j]  (kept f32r for the fp32r
            # prior-add matmul; prior is symmetric so lhsT=prior works)
            fi_col = singles.tile([128, 1], f32)
            nc.sync.dma_start(fi_col, fi_h[:].rearrange("(f o) -> f o", o=1))
            fi_row = singles.tile([128, F], f32)
            nc.sync.dma_start(fi_row, _bc(fi_h[:], 128))
            prior = singles.tile([128, F], f32)
            nc.sync.dma_start(prior, cp_h[:])
            nc.vector.tensor_scalar_mul(prior, prior, fi_col)
            nc.vector.tensor_mul(prior, prior, fi_row)
            prior_r = singles.tile([128, F], f32r)
            nc.vector.tensor_copy(prior_r, prior)

            # 0.5 * xor-block mask (price/vol gating)
            mask05_f = singles.tile([128, F], f32)
            nc.vector.memset(mask05_f[0:64, 0:64], 0.0)
            nc.vector.memset(mask05_f[0:64, 64:128], 0.5)
            nc.vector.memset(mask05_f[64:128, 0:64], 0.5)
            nc.vector.memset(mask05_f[64:128, 64:128], 0.0)
            mask05 = singles.tile([128, F], f32r)
            nc.vector.tensor_copy(mask05, mask05_f)

            # [I I I I]: repeated identity, f32r — one prior matmul covers 4
            # heads' score blocks at fp32r full rate (N=512)
            ident4_r = singles.tile([128, 4, 128], f32r)
            for ii in range(4):
                nc.vector.tensor_copy(ident4_r[:, ii, :], identity)

            # half-mean selector [128, 2]: col0 = first half / 64, col1 = second
            half_sel = singles.tile([128, 2], f32)
            nc.vector.memset(half_sel[0:64, 0:1], 1.0 / 64.0)
            nc.vector.memset(half_sel[64:128, 0:1], 0.0)
            nc.vector.memset(half_sel[0:64, 1:2], 0.0)
            nc.vector.memset(half_sel[64:128, 1:2], 1.0 / 64.0)

            # all of this core's x resident in SBUF: [f, b, d]
            xall = singles.tile([128, bb, D], f32)
            for b in range(bb):
                nc.sync.dma_start(xall[:, b, :], x_h[:][b])

            # ---------------- pass A: price/vol detector ----------------
            comb_dram = dram.tile([bb, 2, D], f32)
            for b in range(bb):
                ps1 = ps_voo.tile([2, D], f32, tag="voo")
                nc.tensor.matmul(ps1, lhsT=half_sel, rhs=xall[:, b, :],
                                 start=True, stop=True)
                pva = sb.tile([2, D], f32, tag="pvA")
                nc.scalar.activation(pva, ps1, AF.Copy)
                nc.sync.dma_start(comb_dram[b], pva)

            combT = singles.tile([128, 2 * DC, bb], f32)
            comb_flat = comb_dram[:].rearrange("b x d -> b (x d)")
            for cc in range(2 * DC):
                nc.sync.dma_start(
                    combT[:, cc, :],
                    comb_flat[:, cc * 128:(cc + 1) * 128].rearrange("b p -> p b"))

            # hT[e, b] = silu(sum_c W1[c, e] * comb[b, c] + b1[e])
            hT_sb = singles.tile([128, DC, bb], f32)
            for ec in range(DC):
                ph = ps_voo.tile([128, bb], f32, tag="voo")
                for cc in range(2 * DC):
                    nc.tensor.matmul(
                        ph, lhsT=w1_s[:, cc, ec * 128:(ec + 1) * 128],
                        rhs=combT[:, cc, :],
                        start=(cc == 0), stop=(cc == 2 * DC - 1))
                # silu(z) = z * sigmoid(z); CoreSim has no native Silu
                sg = sb.tile([128, bb], f32, tag="silu_sg")
                nc.scalar.activation(sg, ph, AF.Sigmoid,
                                     bias=b1_col[:, ec:ec + 1])
                zt = sb.tile([128, bb], f32, tag="silu_z")
                nc.scalar.activation(zt, ph, AF.Identity,
                                     bias=b1_col[:, ec:ec + 1])
                nc.vector.tensor_mul(hT_sb[:, ec, :], sg, zt)

            ps_pv = ps_voo.tile([bb, 1], f32, tag="voo")
            for ec in range(DC):
                nc.tensor.matmul(ps_pv, lhsT=hT_sb[:, ec, :], rhs=w2_s[:, ec, :],
                                 start=(ec == 0), stop=(ec == DC - 1))
            pv_sb = sb.tile([bb, 1], f32, tag="pv")
            nc.scalar.activation(pv_sb, ps_pv, AF.Sigmoid, bias=b2_bc[0:bb])
            pv_dram = dram.tile([bb, 1], f32)
            nc.sync.dma_start(pv_dram, pv_sb)
            pv_bc = singles.tile([128, bb], f32)
            nc.sync.dma_start(pv_bc, _bc(pv_dram[:, 0], 128))

            # ---------------- pass B: attention ----------------
            # batches processed in pairs so the qT/kT matmuls run at N=256
            for bp in range(0, bb, 2):
                pair = (bp, bp + 1)
                # xT chunks for both batches via PE transpose
                xt_ps = ps_t.tile([128, DC, 128], f32, tag="t")
                xt_ps2 = ps_t.tile([128, DC, 128], f32, tag="t")
                xT2 = sb.tile([128, DC, 2, 128], f32, tag="xT")
                for bi, b in enumerate(pair):
                    tp = xt_ps if bi == 0 else xt_ps2
                    for c in range(DC):
                        nc.tensor.transpose(tp[:, c, :],
                                            xall[:, b, c * 128:(c + 1) * 128],
                                            identity)
                    nc.scalar.activation(xT2[:, :, bi, :], tp, AF.Copy)
                # rounded copy for the fp32r v-projection
                xTr = sb.tile([128, DC, 2, 128], f32r, tag="xTr")
                if os.environ.get("KX_XTR", "gps") == "gps":
                    nc.gpsimd.tensor_copy(xTr, xT2)
                else:
                    nc.scalar.activation(xTr, xT2, AF.Copy)

                # qT / kT for the pair: [d', (b f)] layout, bias via ACT evict
                qT2 = sb.tile([128, DC, 2, 128], f32, tag="qT")
                kT2 = sb.tile([128, DC, 2, 128], f32, tag="kT")
                for w_s, b_col, dst in ((wq_r, bq_col, qT2), (wk_r, bk_col, kT2)):
                    ps = ps_qk.tile([128, 2 * 128], f32, tag="qk")
                    for oc in range(DC):
                        osl = slice(oc * 128, (oc + 1) * 128)
                        for kc in range(DC):
                            nc.tensor.matmul(ps,
                                             lhsT=w_s[:, kc, osl],
                                             rhs=xTr[:, kc, :, :],
                                             start=(kc == 0), stop=(kc == DC - 1))
                        nc.scalar.activation(dst[:, oc, :, :], ps, AF.Identity,
                                             bias=b_col[:, oc:oc + 1])

                for bi, b in enumerate(pair):
                    # prior_b = prior + pv_b * mask05 (f32r for the fp32r
                    # prior-add matmul; gpsimd, SBUF only)
                    pb = sb.tile([128, F], f32r, tag="prior_b")
                    nc.gpsimd.tensor_scalar_mul(pb, mask05, pv_bc[:, b:b + 1])
                    nc.gpsimd.tensor_add(pb, pb, prior_r)

                    # v: [f, d'] layout (+ bv), fp32r (4x faster on PE)
                    vps = ps_voo.tile([128, D], f32, tag="voo")
                    for kc in range(DC):
                        nc.tensor.matmul(vps, lhsT=xTr[:, kc, bi, :],
                                         rhs=wv_r[:, kc, :],
                                         start=(kc == 0), stop=False)
                    nc.tensor.matmul(vps, lhsT=ones_row_r, rhs=bv_r,
                                     start=False, stop=True)
                    v_sb = vo.tile([128, D], f32, tag="v")
                    nc.scalar.activation(v_sb, vps, AF.Copy)

                    # scores for all 8 heads into one 2-bank PSUM tile;
                    # prior added via one fp32r N=512 matmul per 4 heads
                    sps = ps_sc.tile([128, H, 128], f32, tag="sc")
                    for g in range(2):
                        for hh in range(4):
                            h = 4 * g + hh
                            c = h // 2
                            po = 64 * (h % 2)
                            nc.tensor.matmul(sps[:, h, :],
                                             lhsT=qT2[po:po + 64, c, bi, :],
                                             rhs=kT2[po:po + 64, c, bi, :],
                                             start=True, stop=False)
                        nc.tensor.matmul(sps[:, 4 * g:4 * g + 4, :],
                                         lhsT=pb, rhs=ident4_r,
                                         start=False, stop=True)
                    # batched exp eviction (one ACT op for all heads)
                    exps_all = hd.tile([128, H, F], f32, tag="exps")
                    nc.scalar.activation(exps_all, sps, AF.Exp)

                    # top-64 threshold per row: 8 segmented displace rounds
                    # over all 8 heads at once ([128, H, w] -> [128, H, w+1]).
                    # Runs on exps (monotone), so the tail slot after round 8
                    # is exp(thresh) directly — no separate eth exp needed.
                    scratch = hd.tile([128, H, 132], f32, tag="scratch")
                    nc.vector._custom_dve(DSEG, out=scratch[:, :, 0:129],
                                          in0=exps_all)
                    for r in range(1, 8):
                        s0 = 8 * r
                        nc.vector._custom_dve(DSEG,
                                              out=scratch[:, :, s0:129],
                                              in0=scratch[:, :, s0:128])

                    # p = exps * (exps >= eth), dn = row-sum (per head)
                    p_all = hd.tile([128, H, F], f32, tag="p")
                    dn_all = hd.tile([128, H], f32, tag="dn")
                    for h in range(H):
                        nc.vector._custom_dve(MES, out=p_all[:, h, :],
                                              in0=exps_all[:, h, :],
                                              in1=exps_all[:, h, :],
                                              s0=scratch[:, h, 128:129],
                                              accum_out=dn_all[:, h:h + 1])
                    rc_all = hd.tile([128, H], f32, tag="rc")
                    nc.vector.reciprocal(rc_all, dn_all)

                    # attn = p / dn (gpsimd, per head scalar; in place)
                    attn_all = p_all
                    for h in range(H):
                        nc.gpsimd.tensor_scalar_mul(attn_all[:, h, :],
                                                    p_all[:, h, :],
                                                    rc_all[:, h:h + 1])

                    # transpose all heads -> one 2-bank PSUM tile, one evict
                    at_ps = ps_at.tile([128, H, 128], f32, tag="at")
                    for h in range(H):
                        nc.tensor.transpose(at_ps[:, h, :], attn_all[:, h, :],
                                            identity)
                    attnT_all = hd.tile([128, H, F], f32, tag="attnT")
                    nc.scalar.activation(attnT_all, at_ps, AF.Copy)

                    # outT[c*128 + po .., q] += v_h-as-lhsT @ attnT_h
                    outT_ps = ps_voo.tile([128, DC, 128], f32, tag="voo")
                    for h in range(H):
                        c = h // 2
                        po = 64 * (h % 2)
                        nc.tensor.matmul(outT_ps[po:po + 64, c, :],
                                         lhsT=v_sb[:, h * HD:(h + 1) * HD],
                                         rhs=attnT_all[:, h, :],
                                         start=True, stop=True,
                                         tile_position=(0, po))
                    # rounded outT for the fp32r output projection
                    outT_sb = sb.tile([128, DC, 128], f32r, tag="outT")
                    nc.scalar.activation(outT_sb, outT_ps, AF.Copy)

                    o_ps = ps_voo.tile([128, D], f32, tag="voo")
                    for cc in range(DC):
                        nc.tensor.matmul(o_ps, lhsT=outT_sb[:, cc, :],
                                         rhs=wo_r[:, cc, :],
                                         start=(cc == 0), stop=False)
                    nc.tensor.matmul(o_ps, lhsT=ones_row_r, rhs=bo_r,
                                     start=False, stop=True)
                    o_sb = vo.tile([128, D], f32, tag="o", bufs=1)
                    nc.scalar.activation(o_sb, o_ps, AF.Copy)
                    nc.sync.dma_start(out_h[:][b], o_sb)

    nc.compile()
    return nc


_NC_CACHE = {}


def kernel(**inputs):
    inputs = {k: np.ascontiguousarray(np.asarray(v, dtype=np.float32))
              for k, v in inputs.items()}
    x = inputs.pop("x")
    assert x.shape == (B, F, D), x.shape

    if BB not in _NC_CACHE:
        _NC_CACHE[BB] = build(BB)
    nc = _NC_CACHE[BB]

    in_maps = []
    for c in range(NCORES):
        m = dict(inputs)
        m["x"] = np.ascontiguousarray(x[c * BB:(c + 1) * BB])
        in_maps.append(m)

    res = run_bass_kernel_spmd(nc, in_maps, core_ids=list(range(NCORES)))
    return np.concatenate([r["out"] for r in res.results], axis=0)


if __name__ == "__main__":
    import reference

    inputs = {k: np.asarray(v) for k, v in reference.setup_inputs().items()}
    got = kernel(**inputs)
    exp = np.asarray(reference.reference(**reference.setup_inputs()))
    err = np.abs(got - exp).max() / np.abs(exp).max()
    print("Relative error:", err)

